# revision 1
# baseline (speedup 1.0000x reference)
"""Low-rank multi-head attention Bass kernel for Trainium2 (8 NeuronCores).

Sharding: (batch, query-block) data parallel. 8 cores = 2 batches x 4 query
blocks. Each core receives the full sequence of its batch, np.roll'ed so its
own query block sits at rows 0:SQ. It computes k1/v1 only for its own
query slice and AllGathers the slices within each 4-core batch group, then
runs attention + output projection for its SQ queries.

Math (per core, per head h):
  q1T = Wq1 @ xq.T            [R, SQ]
  k1T = Wk1 @ xb.T            [R, S]   (k1aug row 32 = ones)
  v1T = Wv1 @ xb.T            [R, S]
  wm_aug = [Wm[h]; b2[h]]     qh rows 0:32 = Wm q1T + b2 x ones (so the k-side
                              bias sum_t k1T[t,j] b2[t] emerges from the k1T
                              contraction), row 32 = q1.b1 + b3
  scoresT[j,i] = k1aug[:,j].T @ qh[:,i]   (K=33; all four bias terms inside)
  attnT = exp(0.125*scoresT)              (no max-subtraction; scores O(1);
                                           no ACT bias -> exp batched per pair)
  aT_aug[65, SQ] = [Vh_h | ones].T @ attnT   (row 64 = softmax denominator)
  wvT[h*64+d, i] = aT[d,i] * (1/denominator[i])
  outT = Wo2T_aug.T @ [o1T; ones], with bv/bo folded into the aug row.

All matmuls run as float32r (1 col/cycle on the PE vs 4 for strict fp32).
"""

import sys

sys.path.insert(0, "/opt/trn_rl_repo")

from contextlib import ExitStack

import numpy as np

import concourse.bass as bass
import concourse.tile as tile
from concourse import bacc
from concourse import mybir
from concourse.masks import make_identity

F32 = mybir.dt.float32
AF = mybir.ActivationFunctionType

H, D, R, N = 20, 64, 32, 1280
NCORES = 8
QP = 4  # query blocks per batch
SCALE = float(D) ** -0.5  # 0.125


def _chunks(total, size):
    out = []
    s = 0
    while s < total:
        out.append((s, min(size, total - s)))
        s += size
    return out


def build_nc(S, SQ, phase=4):
    nc = bacc.Bacc("TRN2", target_bir_lowering=False, debug=False, num_devices=NCORES)

    xb = nc.dram_tensor("xb", [S, N], F32, kind="ExternalInput")
    Wq1 = nc.dram_tensor("Wq1", [R, N], F32, kind="ExternalInput")
    Wq2 = nc.dram_tensor("Wq2", [N, R], F32, kind="ExternalInput")
    bq = nc.dram_tensor("bq", [N], F32, kind="ExternalInput")
    Wk1 = nc.dram_tensor("Wk1", [R, N], F32, kind="ExternalInput")
    Wk2 = nc.dram_tensor("Wk2", [N, R], F32, kind="ExternalInput")
    bk = nc.dram_tensor("bk", [N], F32, kind="ExternalInput")
    Wv1 = nc.dram_tensor("Wv1", [R, N], F32, kind="ExternalInput")
    Wv2 = nc.dram_tensor("Wv2", [N, R], F32, kind="ExternalInput")
    bv = nc.dram_tensor("bv", [N], F32, kind="ExternalInput")
    Wo1 = nc.dram_tensor("Wo1", [R, N], F32, kind="ExternalInput")
    Wo2 = nc.dram_tensor("Wo2", [N, R], F32, kind="ExternalInput")
    bo = nc.dram_tensor("bo", [N], F32, kind="ExternalInput")
    out = nc.dram_tensor("out", [SQ, N], F32, kind="ExternalOutput")

    SCH = _chunks(S, 128)  # sequence chunks (j)
    JSUB = _chunks(S, 512)  # projection free-dim chunks
    OSUB = _chunks(N, 512)  # out-proj free-dim chunks
    ICH = _chunks(SQ, 128)  # output row chunks
    NJ = len(SCH)
    SQP = SQ + (SQ % 2)  # f32r matmuls need an even moving free-dim

    F32R = mybir.dt.float32r

    def evac(dst, src):
        nc.vector.tensor_copy(dst, src)

    eev = [0]

    def evac_early(dst, src):
        # during the early phase ACT is idle; split psum evacuations
        eev[0] += 1
        if eev[0] % 2 == 0:
            nc.scalar.copy(dst, src)
        else:
            nc.vector.tensor_copy(dst, src)

    def mm(out_, lhsT, rhs, **kw):
        nc.tensor.matmul(out_, lhsT, rhs, **kw)

    def tr(out_, in_, ident_ap):
        nc.tensor.matmul(out_, in_, ident_ap, is_transpose=True)

    with tile.TileContext(nc) as tc, ExitStack() as ctx:
        wp = ctx.enter_context(tc.tile_pool(name="wp", bufs=1))
        ps_sc = ctx.enter_context(tc.tile_pool(name="ps_sc", bufs=2, space="PSUM"))
        attn_p = ctx.enter_context(tc.tile_pool(name="attn_p", bufs=6))
        small_p = ctx.enter_context(tc.tile_pool(name="small_p", bufs=4))
        outp = ctx.enter_context(tc.tile_pool(name="outp", bufs=2))
        psum_ctx = ExitStack()
        ps = psum_ctx.enter_context(tc.tile_pool(name="ps_e", bufs=4, space="PSUM"))

        # ---- persistent SBUF tensors ----
        ident = wp.tile([128, 128], F32)
        make_identity(nc, ident[:])
        ones128 = wp.tile([128, 64], F32)
        nc.gpsimd.memset(ones128[:], 1.0)
        onesS = wp.tile([1, S], F32)
        nc.gpsimd.memset(onesS[:], 1.0)
        onesP = wp.tile([128, 20], F32)
        nc.gpsimd.memset(onesP[:], 1.0)

        WqT = wp.tile([128, 320], F32R)  # Wq1.T chunk c at cols 32c:32c+32
        WkT = wp.tile([128, 320], F32R)
        WvT = wp.tile([128, 320], F32R)
        WoT = wp.tile([128, 320], F32R)
        Wq2aug = wp.tile([128, 330], F32)  # [Wq2 | bq] per 64-row head slice
        Wk2aug = wp.tile([128, 330], F32)
        bk_c = wp.tile([128, 10], F32)
        bv_c = wp.tile([128, 12], F32)
        bo_row = wp.tile([1, N], F32)
        Wv2T = wp.tile([32, N], F32R)
        Wo2Ta = wp.tile([33, N], F32R)  # rows 0:32 Wo2.T, row 32 = Wo2@Wo1@bv + bo
        q1Ta = wp.tile([33, SQP], F32R)  # rows 0:32 q1T, row 32 ones
        k1aug = wp.tile([33, S], F32R)  # rows 0:32 k1T, row 32 ones
        v1T = wp.tile([32, S], F32R)
        o1a = wp.tile([33, SQP], F32R)
        t1sb = wp.tile([32, 2], F32R)

        wvT = [wp.tile([128, SQP], F32R, name=f"wvT{c}", tag=f"wvT{c}") for c in range(10)]

        # ================= early phase (xT + projections) =================
        with tc.tile_pool(name="early", bufs=1) as ep, tc.tile_pool(
            name="xin_p", bufs=4
        ) as xin_p, tc.tile_pool(name="wload", bufs=3) as wload:
            # --- k/v weights first: the AllGather should dispatch ASAP ---
            for (w_dram, dstT) in ((Wk1, WkT), (Wv1, WvT)):
                wsb = wload.tile([32, N], F32, tag="wsb")
                nc.sync.dma_start(wsb[:], w_dram[:])
                tp = ps.tile([128, 320], F32, tag="ps")
                for c in range(10):
                    tr(
                        tp[:, 32 * c : 32 * c + 32],
                        wsb[:, 128 * c : 128 * c + 128],
                        ident[:32, :32],
                    )
                evac_early(dstT[:], tp[:])

            # --- x load + transpose into xT (feature-major, queries only;
            # k1/v1 for the rest of the sequence arrive via AllGather) ---
            QCH = _chunks(SQ, 128)
            xT = [ep.tile([128, SQP], F32R, name=f"xT{c}", tag=f"xT{c}") for c in range(10)]
            for g in range(0, len(QCH), 4):
                grp = QCH[g : g + 4]
                xins = []
                for (s0, p) in grp:
                    xin = xin_p.tile([128, N], F32, tag="xin")
                    nc.sync.dma_start(xin[:p, :], xb[s0 : s0 + p, :])
                    xins.append((xin, s0, p))
                for c in range(10):
                    wtot = sum(p for (_, _, p) in xins)
                    tp3 = ps.tile([128, 512], F32, tag="ps")
                    col = 0
                    for (xin, s0, p) in xins:
                        tr(
                            tp3[:, col : col + p],
                            xin[:p, 128 * c : 128 * c + 128],
                            ident[:p, :p],
                        )
                        col += p
                    evac_early(xT[c][:, grp[0][0] : grp[0][0] + wtot], tp3[:, :wtot])

            # local k1/v1 slices (this core's SQ keys), stacked [64, SQP]
            kv_sb = wload.tile([64, SQP], F32R, tag="kv_sb")
            for row0, wT in ((0, WkT), (32, WvT)):
                pps = ps.tile([32, SQP], F32, tag="ps")
                for c in range(10):
                    mm(
                        pps[:],
                        wT[:, 32 * c : 32 * c + 32],
                        xT[c][:],
                        start=(c == 0),
                        stop=(c == 9),
                    )
                evac_early(kv_sb[row0 : row0 + 32, :], pps[:])

            # AllGather the [64, SQ] k/v slices within each 4-core batch
            # group -> full-sequence k1T / v1T in original row order. All
            # remaining weight prep below overlaps the collective latency.
            with tc.tile_pool(name="dramp", bufs=1, space="DRAM") as dramp:
                cc_in = dramp.tile([64, SQ], F32R)
                cc_out = dramp.tile([64 * QP, SQ], F32R)
                nc.sync.dma_start(cc_in[:], kv_sb[:, 0:SQ])
                nc.gpsimd.collective_compute(
                    "AllGather",
                    mybir.AluOpType.bypass,
                    replica_groups=[
                        list(range(g * QP, (g + 1) * QP))
                        for g in range(NCORES // QP)
                    ],
                    ins=[cc_in[:].opt()],
                    outs=[cc_out[:].opt()],
                )
                nc.sync.dma_start(
                    k1aug[0:32, 0:S].rearrange("p (r s) -> p r s", r=QP),
                    cc_out[:].rearrange("(r x) s -> x r s", x=64)[0:32],
                )
                nc.sync.dma_start(
                    v1T[0:32, 0:S].rearrange("p (r s) -> p r s", r=QP),
                    cc_out[:].rearrange("(r x) s -> x r s", x=64)[32:64],
                )

            # --- remaining weights (overlap the collective) ---
            for (w_dram, dstT) in ((Wq1, WqT), (Wo1, WoT)):
                wsb = wload.tile([32, N], F32, tag="wsb")
                nc.sync.dma_start(wsb[:], w_dram[:])
                tp = ps.tile([128, 320], F32, tag="ps")
                for c in range(10):
                    tr(
                        tp[:, 32 * c : 32 * c + 32],
                        wsb[:, 128 * c : 128 * c + 128],
                        ident[:32, :32],
                    )
                evac_early(dstT[:], tp[:])

            for (w_dram, dst) in ((Wq2, Wq2aug), (Wk2, Wk2aug)):
                nc.sync.dma_start(
                    dst[:, :].rearrange("p (c r) -> p c r", r=33)[:, :, 0:32],
                    w_dram[:].rearrange("(c p) r -> p c r", p=128),
                )
            nc.sync.dma_start(
                Wq2aug[:, :].rearrange("p (c r) -> p c r", r=33)[:, :, 32:33],
                bq[:].rearrange("(c p) -> p c", p=128).unsqueeze(2),
            )
            nc.sync.dma_start(
                Wk2aug[:, :].rearrange("p (c r) -> p c r", r=33)[:, :, 32:33],
                bk[:].rearrange("(c p) -> p c", p=128).unsqueeze(2),
            )
            nc.sync.dma_start(bk_c[:], bk[:].rearrange("(c p) -> p c", p=128))
            nc.gpsimd.memset(bv_c[:], 0.0)
            nc.sync.dma_start(bv_c[:, 0:10], bv[:].rearrange("(c p) -> p c", p=128))
            nc.sync.dma_start(bo_row[:], bo[:].unsqueeze(0))

            # Wv2 / Wo2: load [128, 320] (chunk-major), PE-transpose to [32, N]
            for (w_dram, dstT) in ((Wv2, Wv2T), (Wo2, Wo2Ta)):
                wsb2 = wload.tile([128, 320], F32, tag="wsb2")
                nc.sync.dma_start(
                    wsb2[:].rearrange("p (c r) -> p c r", r=32),
                    w_dram[:].rearrange("(c p) r -> p c r", p=128),
                )
                for g0 in range(0, 10, 4):
                    gn = min(4, 10 - g0)
                    tp2 = ps.tile([32, 512], F32, tag="ps")
                    for k in range(gn):
                        c = g0 + k
                        tr(
                            tp2[:, 128 * k : 128 * k + 128],
                            wsb2[:, 32 * c : 32 * c + 32],
                            ident[:],
                        )
                    evac_early(dstT[0:32, 128 * g0 : 128 * (g0 + gn)], tp2[:, : 128 * gn])

            # --- q1 projection ---
            q1ps = ps.tile([32, SQP], F32, tag="ps")
            for c in range(10):
                mm(
                    q1ps[:],
                    WqT[:, 32 * c : 32 * c + 32],
                    xT[c][:, 0:SQP],
                    start=(c == 0),
                    stop=(c == 9),
                )
            evac_early(q1Ta[0:32, :], q1ps[:])
            nc.vector.tensor_copy(q1Ta[32:33, :], onesS[:, 0:SQP])
            nc.vector.tensor_copy(k1aug[32:33, :], onesS[:])

            # --- bo_eff into Wo2Ta row 32 ---
            bv_cr = wload.tile([128, 12], F32R, tag="bv_cr")
            nc.vector.tensor_copy(bv_cr[:], bv_c[:])
            t1ps = ps.tile([32, 2], F32, tag="ps")
            for c in range(10):
                mm(
                    t1ps[:],
                    WoT[:, 32 * c : 32 * c + 32],
                    bv_cr[:, c : c + 2],
                    start=(c == 0),
                    stop=(c == 9),
                )
            evac(t1sb[:], t1ps[:])
            for (n0, nw) in OSUB:
                beps = ps.tile([1, 512], F32, tag="ps")
                mm(beps[:, :nw], t1sb[:, 0:1], Wo2Ta[0:32, n0 : n0 + nw])
                nc.vector.tensor_add(
                    Wo2Ta[32:33, n0 : n0 + nw], beps[:, :nw], bo_row[:, n0 : n0 + nw]
                )

        # ================= Vh_aug construction =================
        if phase < 1:
            nc.sync.dma_start(out[0:128, :], q1Ta[0:33, 0:N] if False else k1aug[0:33, 0:N])
        late = ctx.enter_context(tc.tile_pool(name="late", bufs=1))
        Vh = [
            late.tile([128, H * 65], F32R, name=f"Vh{j}", tag=f"Vh{j}")
            for j in range(NJ)
        ]
        for j, (j0, p) in enumerate(SCH):
            if phase < 2:
                break
            ones_ap = Vh[j][:p, :].rearrange("p (h c) -> p h c", c=65)[:, :, 64:65]
            nc.vector.tensor_copy(ones_ap, onesP[:p, :].unsqueeze(2))
            for (n0, nw) in OSUB:
                vps = ps.tile([128, 512], F32, tag="ps")
                mm(vps[:p, :nw], v1T[:, j0 : j0 + p], Wv2T[:, n0 : n0 + nw])
                h0 = n0 // 64
                hn = nw // 64
                dst = Vh[j][:p, 65 * h0 : 65 * (h0 + hn)].rearrange(
                    "p (h c) -> p h c", c=65
                )[:, :, 0:64]
                src = vps[:p, :nw].rearrange("p (h c) -> p h c", c=64)
                evac(dst, src)

        # ---- per-head constants, hoisted out of the attention loop ----
        qh_all = []
        for h in range(H):
            hc, half = h // 2, (h % 2) * 64
            # wm_aug [33, 33]:
            #   cols 0:32: rows 0:32 = Wm[h] = Wq2h.T @ Wk2h, row 32 = b2[h]
            #   col 32   = [b1; b3]  (b1 = Wq2h.T bk_h, b3 = bq.bk)
            # One f32r matmul against q1Ta then yields all 33 qh rows at
            # partition 0 (f32r matmuls reject partition-offset outputs).
            wmps = ps.tile([33, 33], F32, tag="ps")
            mm(
                wmps[0:32, 0:32],
                Wq2aug[half : half + 64, 33 * hc : 33 * hc + 32],
                Wk2aug[half : half + 64, 33 * hc : 33 * hc + 32],
            )
            mm(
                wmps[32:33, 0:32],
                Wq2aug[half : half + 64, 33 * hc + 32 : 33 * hc + 33],
                Wk2aug[half : half + 64, 33 * hc : 33 * hc + 32],
                skip_group_check=True,
            )
            mm(
                wmps[0:33, 32:33],
                Wq2aug[half : half + 64, 33 * hc : 33 * hc + 33],
                bk_c[half : half + 64, hc : hc + 1],
                skip_group_check=True,
            )
            wm = small_p.tile([33, 33], F32R, tag="wm")
            evac(wm[:], wmps[:])

            # qh: rows 0:32 = Wm q1T + b2 x ones, row 32 = q1.b1 + b3
            qhps = ps.tile([33, SQP], F32, tag="ps")
            mm(qhps[:], wm[:], q1Ta[:])
            qh = wp.tile([33, SQP], F32R, name=f"qh{h}", tag=f"qh{h}")
            evac(qh[:], qhps[:])
            qh_all.append(qh)

        # ---- switch PSUM pools: early pool out, 4 accumulator banks in ----
        psum_ctx.close()
        psum_ctx = ExitStack()
        ps_acc = psum_ctx.enter_context(
            tc.tile_pool(name="ps_acc", bufs=1, space="PSUM")
        )

        # ================= attention core (head pairs) =================
        for hp in range(H // 2 if phase >= 3 else 0):
            heads = (2 * hp, 2 * hp + 1)
            accs = [
                ps_acc.tile([65, SQP], F32, tag="acc", name=f"acc{h}", bufs=3)
                for h in heads
            ]

            # software-pipelined: scores(j+1) issues on the PE before
            # attnV(j), so the PE never idles waiting on exp(j)
            scs = {}

            def do_scores(j):
                j0, p = SCH[j]
                sc = ps_sc.tile([128, 1024], F32, tag="sc", name=f"sc{j}")
                mm(sc[:p, 0:SQP], k1aug[:, j0 : j0 + p], qh_all[heads[0]][:])
                mm(sc[:p, 512 : 512 + SQP], k1aug[:, j0 : j0 + p], qh_all[heads[1]][:])
                scs[j] = sc

            do_scores(0)
            for j, (j0, p) in enumerate(SCH):
                sc = scs.pop(j)
                at2 = attn_p.tile([128, 2 * SQP], F32R, tag="at")
                nc.scalar.activation(
                    at2[:p, :].rearrange("p (b c) -> p b c", c=SQP),
                    sc[:p, :].rearrange("p (b c) -> p b c", c=512)[:, :, 0:SQP],
                    AF.Exp,
                    scale=SCALE,
                )
                if j + 1 < NJ:
                    do_scores(j + 1)
                for idx, h in enumerate(heads):
                    mm(
                        accs[idx][:],
                        Vh[j][:p, 65 * h : 65 * h + 65],
                        at2[:p, idx * SQP : (idx + 1) * SQP],
                        start=(j == 0),
                        stop=(j == NJ - 1),
                    )

            # normalize: wvT rows [64h:64h+64] = acc[0:64] * (1/acc[64])
            for idx, h in enumerate(heads):
                acc = accs[idx]
                half = (h % 2) * 64
                rrs = small_p.tile([65, SQP], F32, tag="rrs")
                nc.vector.reciprocal(rrs[64:65, :], acc[64:65, :])
                bc = ps_acc.tile([64, SQP], F32, tag="bc", bufs=1)
                mm(bc[:], ones128[64:65, :], rrs[64:65, :])
                bc_sb = small_p.tile([64, SQP], F32, tag="bc_sb")
                nc.vector.tensor_copy(bc_sb[:], bc[:])
                nc.vector.tensor_mul(
                    wvT[h // 2][half : half + 64, :], bc_sb[:], acc[0:64, :]
                )

        # ================= output projection =================
        psum_ctx.close()
        psum_ctx = ExitStack()
        ps = psum_ctx.enter_context(tc.tile_pool(name="ps_f", bufs=3, space="PSUM"))
        if phase < 4:
            psum_ctx.close()
            nc.compile()
            return nc
        o1ps = ps.tile([32, SQP], F32, tag="ps")
        for c in range(10):
            mm(
                o1ps[:],
                WoT[:, 32 * c : 32 * c + 32],
                wvT[c][:],
                start=(c == 0),
                stop=(c == 9),
            )
        evac_early(o1a[0:32, :], o1ps[:])
        nc.vector.tensor_copy(o1a[32:33, :], onesS[:, 0:SQP])

        for (i0, iw) in ICH:
            osb = outp.tile([128, N], F32, tag="osb")
            for (n0, nw) in OSUB:
                fps = ps.tile([128, 512], F32, tag="ps")
                mm(fps[:iw, :nw], o1a[:, i0 : i0 + iw], Wo2Ta[:, n0 : n0 + nw])
                evac_early(osb[:iw, n0 : n0 + nw], fps[:iw, :nw])
            nc.sync.dma_start(out[i0 : i0 + iw, :], osb[:iw, :])
        psum_ctx.close()

    nc.compile()
    return nc


_NC_CACHE = {}


def _get_nc(S, SQ):
    key = (S, SQ)
    if key not in _NC_CACHE:
        _NC_CACHE[key] = build_nc(S, SQ)
    return _NC_CACHE[key]


def kernel(**inputs):
    from concourse.bass_utils import run_bass_kernel_spmd

    x = np.asarray(inputs["x"], dtype=np.float32)
    B, S, n = x.shape
    assert n == N and B * QP == NCORES
    SQ = S // QP
    nc = _get_nc(S, SQ)

    wnames = [
        "Wq1", "Wq2", "bq", "Wk1", "Wk2", "bk",
        "Wv1", "Wv2", "bv", "Wo1", "Wo2", "bo",
    ]
    weights = {k: np.ascontiguousarray(np.asarray(inputs[k], dtype=np.float32)) for k in wnames}

    in_maps = []
    for core in range(NCORES):
        b, qi = divmod(core, QP)
        xbv = np.ascontiguousarray(np.roll(x[b], -SQ * qi, axis=0))
        m = {"xb": xbv}
        m.update(weights)
        in_maps.append(m)

    res = run_bass_kernel_spmd(nc, in_maps, core_ids=list(range(NCORES)))
    outs = res.results if hasattr(res, "results") else res

    out = np.zeros((B, S, N), dtype=np.float32)
    for core in range(NCORES):
        b, qi = divmod(core, QP)
        out[b, SQ * qi : SQ * (qi + 1), :] = outs[core]["out"]
    return out

